# revision 1
# baseline (speedup 1.0000x reference)
"""MultiHeadDiffAttention Trainium2 kernel (8 NeuronCores).

Sharding: data-parallel over batch (B=2 -> 2 groups of 4 cores), tensor-parallel
over heads within a group (16 heads -> 4 heads/core). Each core computes its
heads' attention output transposed [256, S], applies its Wo row-slice to get a
partial [S, D] output, then a 4-core ReduceScatter sums partials and scatters
S; each core LayerNorms its S/4 slice and writes [512, 1024].

All matmuls run in float32r (fp32 storage, ~1e-4 matmul error, full PE speed).
Softmax is computed unnormalized in transposed [k, q] layout; the row-sums ride
along the P^T@V matmul via a ones column appended to V, and normalization plus
the lambda-combine happen on the small [65, S] outputs.
"""

import math
from contextlib import ExitStack

import numpy as np

import concourse.bass as bass
import concourse.mybir as mybir
import concourse.tile as tile
from concourse import bacc
from concourse import bass_utils

F32 = mybir.dt.float32
F32R = mybir.dt.float32r
BF16 = mybir.dt.bfloat16

B = 2
S = 2048
D = 1024
NH = 16
HD = 64
N_CORES = 8
NH_LOC = NH // (N_CORES // B)  # 4 heads per core
DQ = NH_LOC * 2 * HD  # 512 local q/k projection width
DV = NH_LOC * HD  # 256 local v projection width
LAYER_IDX = 12
LAMBDA_INIT = 0.8 - 0.6 * math.exp(-0.3 * (LAYER_IDX - 1))
LN_EPS = 1e-5
SCALE = HD ** (-0.5)

KC = D // 128  # 8 contraction chunks for projections
SB = S // 128  # 16 S-blocks / k-chunks
QT_TILES = S // 512  # 4 q tiles of 512
NT = D // 512  # 2 output n-tiles for Wo

_CACHE = {}
FAST_DEFAULT = False


def _build(repeat=1, single=False, fast=False, nocc=False):
    # fast=True: attention inner loop (scores, softmax weights, PV, Wo) in
    # bf16 so PSUM evacuations move to DVE; projections stay f32r.
    ADT = BF16 if fast else F32R
    nc = bacc.Bacc("TRN2", target_bir_lowering=False, debug=False,
                   num_devices=1 if single else N_CORES)

    xT = nc.dram_tensor("xT", [D, S], F32R, kind="ExternalInput").ap()
    wq = nc.dram_tensor("wq", [D, DQ], F32R, kind="ExternalInput").ap()
    wk = nc.dram_tensor("wk", [D, DQ], F32R, kind="ExternalInput").ap()
    wv = nc.dram_tensor("wv", [D, DV], F32R, kind="ExternalInput").ap()
    wo = nc.dram_tensor("wo", [DV, D], F32R, kind="ExternalInput").ap()
    lam = nc.dram_tensor("lam", [1, 1], F32, kind="ExternalInput").ap()
    gamma = nc.dram_tensor("gamma", [1, D], F32, kind="ExternalInput").ap()
    beta = nc.dram_tensor("beta", [1, D], F32, kind="ExternalInput").ap()
    ones4 = nc.dram_tensor("ones4", [1, NH_LOC], F32, kind="ExternalInput").ap()
    out = nc.dram_tensor("out", [S // 4, D], F32, kind="ExternalOutput").ap()

    with tile.TileContext(nc) as tc, ExitStack() as ctx:
        sb = ctx.enter_context(tc.tile_pool(name="sb", bufs=1))
        ps = ctx.enter_context(tc.tile_pool(name="ps", bufs=1, space="PSUM"))
        dram = ctx.enter_context(tc.tile_pool(name="dram", bufs=1, space="DRAM"))

        # ---- constants ----
        lam_sb = sb.tile([128, 1], F32, tag="lam")
        nc.sync.dma_start(out=lam_sb, in_=lam.to_broadcast([128, 1]))
        gamma_sb = sb.tile([128, D], F32, tag="gamma")
        nc.sync.dma_start(out=gamma_sb, in_=gamma.to_broadcast([128, D]))
        beta_sb = sb.tile([128, D], F32, tag="beta")
        nc.sync.dma_start(out=beta_sb, in_=beta.to_broadcast([128, D]))
        ones4_sb = sb.tile([128, NH_LOC], F32, tag="ones4")
        nc.sync.dma_start(out=ones4_sb, in_=ones4.to_broadcast([128, NH_LOC]))
        eps_sb = sb.tile([128, 1], F32, tag="eps")
        nc.vector.memset(eps_sb, LN_EPS)

        def emit_body():
            # ---- weight / x loads ----
            # x split into 4 column slices per 128-row chunk so the first
            # projections can start at ~1/4 of the x transfer.
            xr = xT.rearrange("(c p) s -> c p s", p=128)
            xc = []
            for c in range(KC):
                t = sb.tile([128, S], F32R, tag=f"xT{c}", name=f"xc{c}")
                for q in range(QT_TILES):
                    qs = slice(q * 512, (q + 1) * 512)
                    nc.sync.dma_start(out=t[:, qs], in_=xr[c][:, qs])
                xc.append(t)
            wqr = wq.rearrange("(c p) m -> c p m", p=128)
            wkr = wk.rearrange("(c p) m -> c p m", p=128)
            wvr = wv.rearrange("(c p) m -> c p m", p=128)
            wqc, wkc, wvc = [], [], []
            for c in range(KC):
                tq = sb.tile([128, DQ], F32R, tag=f"wq{c}", name=f"wqc{c}")
                nc.sync.dma_start(out=tq, in_=wqr[c])
                wqc.append(tq)
                tk = sb.tile([128, DQ], F32R, tag=f"wk{c}", name=f"wkc{c}")
                nc.sync.dma_start(out=tk, in_=wkr[c])
                wkc.append(tk)
                tv = sb.tile([128, DV], F32R, tag=f"wv{c}", name=f"wvc{c}")
                nc.sync.dma_start(out=tv, in_=wvr[c])
                wvc.append(tv)
            wor = wo.rearrange("(c p) m -> c p m", p=128)
            woc = []
            for c in range(DV // 128):
                if fast:
                    tmp = sb.tile([128, D], F32, tag="wotmp", name=f"wotmp{c}")
                    nc.sync.dma_start(out=tmp, in_=wor[c].bitcast(F32))
                    t = sb.tile([128, D], BF16, tag=f"wo{c}", name=f"woc{c}")
                    nc.vector.tensor_copy(t, tmp)
                else:
                    t = sb.tile([128, D], F32R, tag=f"wo{c}", name=f"woc{c}")
                    nc.sync.dma_start(out=t, in_=wor[c])
                woc.append(t)

            # ---- V projection -> Vones [128, NH_LOC, HD+1] per S-chunk ----
            vones = []
            for c in range(SB):
                t = sb.tile([128, NH_LOC, HD + 1], ADT, tag=f"vo{c}", name=f"vones{c}")
                vones.append(t)
            for c in range(SB):
                pv = ps.tile([128, DV], F32, tag="proj", bufs=2, name="pv")
                for d in range(KC):
                    nc.tensor.matmul(pv, xc[d][:, c * 128:(c + 1) * 128], wvc[d],
                                     start=(d == 0), stop=(d == KC - 1))
                pvr = pv.rearrange("p (h v) -> p h v", h=NH_LOC)
                if fast:
                    nc.vector.tensor_copy(vones[c][:, :, 0:HD], pvr)
                    nc.vector.memset(vones[c][:, :, HD:HD + 1], 1.0)
                else:
                    nc.scalar.copy(vones[c][:, :, 0:HD], pvr)
                    nc.scalar.copy(vones[c][:, :, HD:HD + 1],
                                   ones4_sb.rearrange("p (a o) -> p a o", o=1))

            # ---- output-transposed accumulator [2][128, S] (4 heads) ----
            otc = []
            for c in range(2):
                t = sb.tile([128, S], ADT, tag=f"ot{c}", name=f"otc{c}")
                otc.append(t)
            partial = dram.tile([S, D], F32, name="partial")

            # ---- per-head: QT/KT projection then attention ----
            for h in range(NH_LOC):
                qt_h = sb.tile([128, S], ADT, tag="qt", bufs=1, name="qt_h")
                kt_h = sb.tile([128, S], ADT, tag="kt", bufs=1, name="kt_h")
                hc0 = h * 128
                for qt in range(QT_TILES):
                    sl = slice(qt * 512, (qt + 1) * 512)
                    pq = ps.tile([128, 512], F32, tag="proj", bufs=2, name="pq")
                    for d in range(KC):
                        nc.tensor.matmul(pq, wqc[d][:, hc0:hc0 + 128], xc[d][:, sl],
                                         start=(d == 0), stop=(d == KC - 1))
                    if fast:
                        nc.vector.tensor_copy(qt_h[:, sl], pq)
                    else:
                        nc.scalar.copy(qt_h[:, sl], pq)
                    pk = ps.tile([128, 512], F32, tag="proj", bufs=2, name="pk")
                    for d in range(KC):
                        nc.tensor.matmul(pk, wkc[d][:, hc0:hc0 + 128], xc[d][:, sl],
                                         start=(d == 0), stop=(d == KC - 1))
                    if fast:
                        nc.vector.tensor_copy(kt_h[:, sl], pk)
                    else:
                        nc.scalar.copy(kt_h[:, sl], pk)

                for qt in range(QT_TILES):
                    qsl = slice(qt * 512, (qt + 1) * 512)
                    po1 = ps.tile([HD + 1, 512], F32, tag="po1", bufs=1, name="po1")
                    po2 = ps.tile([HD + 1, 512], F32, tag="po2", bufs=1, name="po2")
                    for c in range(SB):
                        ksl = slice(c * 128, (c + 1) * 128)
                        # A1 | A2 share one 2-bank psum tile -> single exp
                        pa = ps.tile([128, 1024], F32, tag="pa", bufs=2, name="pa")
                        nc.tensor.matmul(pa[:, 0:512], kt_h[0:HD, ksl],
                                         qt_h[0:HD, qsl])
                        nc.tensor.matmul(pa[:, 512:1024], kt_h[HD:128, ksl],
                                         qt_h[HD:128, qsl])
                        e12 = sb.tile([128, 1024], ADT, tag="e12", bufs=2, name="e12")
                        nc.scalar.activation(out=e12, in_=pa,
                                             func=mybir.ActivationFunctionType.Exp,
                                             scale=SCALE)
                        nc.tensor.matmul(po1, vones[c][:, h, :], e12[:, 0:512],
                                         start=(c == 0), stop=(c == SB - 1))
                        nc.tensor.matmul(po2, vones[c][:, h, :], e12[:, 512:1024],
                                         start=(c == 0), stop=(c == SB - 1))
                    # normalize + lambda combine on [65, 512]; row 64 = sums
                    o1sb = sb.tile([HD + 1, 512], F32, tag="o1sb", bufs=1, name="o1sb")
                    o2sb = sb.tile([HD + 1, 512], F32, tag="o2sb", bufs=1, name="o2sb")
                    nc.vector.tensor_copy(o1sb, po1)
                    nc.vector.tensor_copy(o2sb, po2)
                    srow1 = sb.tile([1, 512], F32, tag="srow1", bufs=1, name="srow1")
                    srow2 = sb.tile([1, 512], F32, tag="srow2", bufs=1, name="srow2")
                    nc.vector.tensor_copy(srow1, o1sb[HD:HD + 1, :])
                    nc.vector.tensor_copy(srow2, o2sb[HD:HD + 1, :])
                    r1 = sb.tile([HD, 512], F32, tag="r1", bufs=1, name="r1")
                    r2 = sb.tile([HD, 512], F32, tag="r2", bufs=1, name="r2")
                    nc.gpsimd.partition_broadcast(r1, srow1, channels=HD)
                    nc.gpsimd.partition_broadcast(r2, srow2, channels=HD)
                    nc.vector.reciprocal(r1, r1)
                    nc.vector.reciprocal(r2, r2)
                    nc.vector.tensor_scalar_mul(r2, r2, lam_sb[0:HD, :])
                    nc.vector.tensor_mul(r1, o1sb[0:HD, :], r1)
                    nc.vector.tensor_mul(r2, o2sb[0:HD, :], r2)
                    rb = (h % 2) * HD
                    if fast:
                        nc.vector.tensor_sub(otc[h // 2][rb:rb + HD, qsl], r1, r2)
                    else:
                        nc.vector.tensor_sub(r1, r1, r2)
                        nc.scalar.copy(otc[h // 2][rb:rb + HD, qsl], r1)

                    # Wo partial for this q-tile's S-blocks once the last
                    # head's combine has landed (earlier heads already wrote).
                    if h == NH_LOC - 1:
                        for sblk in range(qt * 4, qt * 4 + 4):
                            csl = slice(sblk * 128, (sblk + 1) * 128)
                            for nt in range(NT):
                                nsl = slice(nt * 512, (nt + 1) * 512)
                                pw = ps.tile([128, 512], F32, tag="proj",
                                             bufs=2, name="pw")
                                nc.tensor.matmul(pw, otc[0][:, csl],
                                                 woc[0][:, nsl],
                                                 start=True, stop=False)
                                nc.tensor.matmul(pw, otc[1][:, csl],
                                                 woc[1][:, nsl],
                                                 start=False, stop=True)
                                wout = sb.tile([128, 512], F32, tag="wout",
                                               bufs=2, name="wout")
                                nc.vector.tensor_copy(wout, pw)
                                nc.sync.dma_start(out=partial[csl, nsl],
                                                  in_=wout)

            # ---- ReduceScatter over 4-core group ----
            red = dram.tile([S // 4, D], F32, name="red")
            if single or nocc:
                nc.sync.dma_start(out=red[:, :], in_=partial[0:S // 4, :])
            else:
                nc.gpsimd.collective_compute(
                    "ReduceScatter",
                    mybir.AluOpType.add,
                    replica_groups=[[0, 1, 2, 3], [4, 5, 6, 7]],
                    ins=[partial.opt()],
                    outs=[red.opt()],
                )

            # ---- LayerNorm on local S/4 rows ----
            for i in range(S // 4 // 128):
                rsl = slice(i * 128, (i + 1) * 128)
                xt = sb.tile([128, D], F32, tag="lnx", bufs=1, name="xt")
                nc.sync.dma_start(out=xt, in_=red[rsl, :])
                xrr = xt.rearrange("p (a b) -> p a b", b=512)
                st = sb.tile([128, 2, 6], F32, tag="st", bufs=2, name="st")
                nc.vector.bn_stats(out=st[:, 0, :], in_=xrr[:, 0, :])
                nc.vector.bn_stats(out=st[:, 1, :], in_=xrr[:, 1, :])
                mv = sb.tile([128, 2], F32, tag="mv", bufs=2, name="mv")
                nc.vector.bn_aggr(out=mv, in_=st)
                rstd = sb.tile([128, 1], F32, tag="rstd", bufs=2, name="rstd")
                nc.scalar.activation(out=rstd, in_=mv[:, 1:2],
                                     func=mybir.ActivationFunctionType.Sqrt,
                                     bias=eps_sb, scale=1.0)
                nc.vector.reciprocal(rstd, rstd)
                ot = sb.tile([128, D], F32, tag="lno", bufs=1, name="ot")
                nc.vector.tensor_scalar(ot, xt, mv[:, 0:1], rstd,
                                        op0=mybir.AluOpType.subtract,
                                        op1=mybir.AluOpType.mult)
                nc.vector.tensor_mul(ot, ot, gamma_sb)
                nc.vector.tensor_add(ot, ot, beta_sb)
                nc.sync.dma_start(out=out[rsl, :], in_=ot)


        for _rep in range(repeat):
            emit_body()

    nc.compile()
    return nc


def _shard(inputs):
    x = np.asarray(inputs["x"], dtype=np.float32)
    Wq = np.asarray(inputs["Wq"], dtype=np.float32)
    Wk = np.asarray(inputs["Wk"], dtype=np.float32)
    Wv = np.asarray(inputs["Wv"], dtype=np.float32)
    Wo = np.asarray(inputs["Wo"], dtype=np.float32)
    gamma = np.asarray(inputs["gamma"], dtype=np.float32).reshape(1, D)
    beta = np.asarray(inputs["beta"], dtype=np.float32).reshape(1, D)
    lq1 = np.asarray(inputs["lambda_q1"], dtype=np.float32)
    lk1 = np.asarray(inputs["lambda_k1"], dtype=np.float32)
    lq2 = np.asarray(inputs["lambda_q2"], dtype=np.float32)
    lk2 = np.asarray(inputs["lambda_k2"], dtype=np.float32)
    lam = (np.exp(np.sum(lq1 * lk1, dtype=np.float32), dtype=np.float32)
           - np.exp(np.sum(lq2 * lk2, dtype=np.float32), dtype=np.float32)
           + np.float32(LAMBDA_INIT)).reshape(1, 1).astype(np.float32)
    ones4 = np.ones((1, NH_LOC), dtype=np.float32)

    wq_h = Wq.reshape(D, NH, 2 * HD)
    wk_h = Wk.reshape(D, NH, 2 * HD)
    wv_h = Wv.reshape(D, NH, HD)
    wo_h = Wo.reshape(NH, HD, D)

    xTs = [np.ascontiguousarray(x[b].T) for b in range(B)]
    in_maps = []
    for c in range(N_CORES):
        b = c // (N_CORES // B)
        hg = c % (N_CORES // B)
        hs = slice(hg * NH_LOC, (hg + 1) * NH_LOC)
        in_maps.append({
            "xT": xTs[b],
            "wq": np.ascontiguousarray(wq_h[:, hs, :].reshape(D, DQ)),
            "wk": np.ascontiguousarray(wk_h[:, hs, :].reshape(D, DQ)),
            "wv": np.ascontiguousarray(wv_h[:, hs, :].reshape(D, DV)),
            "wo": np.ascontiguousarray(wo_h[hs].reshape(DV, D)),
            "lam": lam,
            "gamma": gamma,
            "beta": beta,
            "ones4": ones4,
        })
    return in_maps


def run_all(trace=False, repeat=1, fast=FAST_DEFAULT, nocc=False, **inputs):
    key = (repeat, fast, nocc)
    if key not in _CACHE:
        _CACHE[key] = _build(repeat=repeat, fast=fast, nocc=nocc)
    nc = _CACHE[key]
    in_maps = _shard(inputs)
    res = bass_utils.run_bass_kernel_spmd(
        nc, in_maps, core_ids=list(range(N_CORES)), trace=trace)
    out = np.empty((B, S, D), dtype=np.float32)
    for c in range(N_CORES):
        b = c // (N_CORES // B)
        r = c % (N_CORES // B)
        out[b, r * (S // 4):(r + 1) * (S // 4), :] = res.results[c]["out"]
    return out, res


def kernel(**inputs):
    out, _ = run_all(trace=False, **inputs)
    return out

